# revision 3
# baseline (speedup 1.0000x reference)
"""Trainium2 Bass kernel for squared-Euclidean distance to prototypes (retrieval_knn).

out[b,h,w,u] = ||x[b,h,w,:] - w[u,:]||^2 = x2 - 2*x.w^T + w2

Strategy: data-parallel over the flattened row dim (B*H*W = 524288 rows) across
8 NeuronCores, 65536 rows per core, prototypes replicated. Forward only, so no
collectives.

Per-core device kernel (per 1024-row chunk, laid out [128 part, 8 blk, 64 d]):
  1. DMA chunk in (contiguous 2KB/partition).
  2. DVE tensor_tensor_reduce per block -> x2 per row (fp32).
  3. PE transposes: pairs of [128,64] blocks as one [128,128] -> psum [128,128]
     (d on partitions), copied to SBUF (split ACT/DVE).
  4. PE matmul per block: psum_out[:, c*64:+64] = xT.T @ (-2 w^T)  (fp32).
  5. One small augmented matmul adds x2[row] + w2[u] into the same PSUM bank:
     lhsT = [x2 rows (8); ones] (from a [128,9] PE transpose), rhs = EW const.
  6. Epilogue copy psum -> sbuf, DMA out.
"""

import os
import sys
from contextlib import ExitStack

import numpy as np

sys.path.insert(0, "/opt/trn_rl_repo")

import concourse.bass as bass
import concourse.tile as tile
from concourse import bacc, mybir
from concourse._compat import with_exitstack

# Problem geometry (hardcoded per contest contract)
B, H, W_DIM, D = 16, 128, 256, 64
UNITS = 64
N_CORES = 8
N_TOTAL = B * H * W_DIM              # 524288 rows
N_CORE = N_TOTAL // N_CORES          # 65536 rows per core
P = 128                              # partitions
CBLK = 8                             # 64-d blocks per chunk
CHUNK_ROWS = P * CBLK                # 1024 rows per chunk
N_CHUNKS = N_CORE // CHUNK_ROWS      # 64

FP = mybir.dt.float32
# engine-balance tuning knobs (numerics-exact splits)
SQ_DVE_BLOCKS = 4    # of CBLK blocks of the x^2 square on DVE; rest on ACT
EPI_ACT_COLS = 256   # of 512 epilogue cols copied by ACT; rest on DVE


@with_exitstack
def _knn_tile_kernel(ctx: ExitStack, tc: tile.TileContext, n_rows: int,
                     ew_mm_f32r: bool = False, repeat: int = 1,
                     hw_repeat: int = 1, pair_mm_bf16: bool = False,
                     xbar_transpose: bool = False, sq_gpsimd: bool = False,
                     big_bufs: bool = False, sq_split: bool = False,
                     dma_split: bool = False, cblk: int = CBLK,
                     psum_rebal: bool = False, red_split: bool = False,
                     ew_last: bool = False):
    """Emit the per-core program. Tensors are declared on tc.nc.

    repeat: unroll the whole chunk loop N times (python-level, for timing).
    hw_repeat: wrap the body in a hardware For_i loop re-processing the same
    data N times (timing only; slope over hw_repeat isolates device time
    from the ~100ms axon dispatch overhead).
    """
    nc = tc.nc
    if xbar_transpose:
        assert pair_mm_bf16, "xbar transpose path requires bf16 pair matmuls"

    chunk_rows = P * cblk
    n_chunks = n_rows // chunk_rows
    assert n_rows % chunk_rows == 0
    nbank = (cblk * UNITS + 511) // 512  # psum banks per chunk

    xin = nc.dram_tensor("xc", [n_rows, D], FP, kind="ExternalInput").ap()
    out = nc.dram_tensor("outc", [n_rows, UNITS], FP, kind="ExternalOutput").ap()
    w_dt = mybir.dt.bfloat16 if pair_mm_bf16 else FP
    wn2t = nc.dram_tensor("wn2t", [P, 2 * UNITS], w_dt, kind="ExternalInput").ap()
    ew_dt = mybir.dt.float32r if ew_mm_f32r else FP
    ew = nc.dram_tensor("ew", [cblk + 1, cblk * UNITS], ew_dt, kind="ExternalInput").ap()
    ident = nc.dram_tensor("ident", [P, P], FP, kind="ExternalInput").ap()

    xv = xin.rearrange("(t p c) d -> t p (c d)", p=P, c=cblk)
    ov = out.rearrange("(t p c) u -> t p (c u)", p=P, c=cblk)

    consts = ctx.enter_context(tc.tile_pool(name="consts", bufs=1))
    sb_w = consts.tile([P, 2 * UNITS], w_dt)
    nc.sync.dma_start(sb_w, wn2t)
    sb_ew = consts.tile([cblk + 1, cblk * UNITS], ew_dt)
    nc.sync.dma_start(sb_ew, ew)
    sb_id = consts.tile([P, P], FP)
    nc.sync.dma_start(sb_id, ident)

    nb = 5 if big_bufs else 3
    xpool = ctx.enter_context(tc.tile_pool(name="xin", bufs=nb))
    xbpool = ctx.enter_context(tc.tile_pool(name="xbf", bufs=3))
    sqpool = ctx.enter_context(tc.tile_pool(name="sq", bufs=4 if big_bufs else 2))
    xtpool = ctx.enter_context(tc.tile_pool(name="xt", bufs=nb))
    x2pool = ctx.enter_context(tc.tile_pool(name="x2", bufs=nb))
    opool = ctx.enter_context(tc.tile_pool(name="osb", bufs=nb))
    pst_bufs = (3 if big_bufs else 2) if cblk <= 8 else 3
    pso_bufs = (3 if big_bufs else 2) if cblk <= 8 else 2
    psr_bufs = 2 if cblk <= 8 else 1
    if psum_rebal:
        pst_bufs, pso_bufs, psr_bufs = 4, 3, 1
    ps_t = ctx.enter_context(tc.tile_pool(name="ps_t", bufs=pst_bufs, space="PSUM"))
    ps_o = ctx.enter_context(tc.tile_pool(name="ps_o", bufs=pso_bufs, space="PSUM"))
    ps_r = ctx.enter_context(tc.tile_pool(name="ps_r", bufs=psr_bufs, space="PSUM"))

    NPAIR = CBLK // 2  # transpose pairs per chunk

    import contextlib
    loop_cm = (
        tc.For_i(0, hw_repeat, 1) if hw_repeat > 1 else contextlib.nullcontext()
    )
    with loop_cm:
      for t in [t for _ in range(repeat) for t in range(n_chunks)]:
        npair = cblk // 2
        x_sb = xpool.tile([P, cblk, D], FP)
        xsrc = xv[t].rearrange("p (c d) -> p c d", c=cblk)
        if dma_split:
            hh = cblk // 2
            nc.sync.dma_start(x_sb[:, :hh, :], xsrc[:, :hh, :])
            nc.sync.dma_start(x_sb[:, hh:, :], xsrc[:, hh:, :])
        else:
            nc.sync.dma_start(x_sb, xsrc)

        # x2 per row (fp32): square split DVE/ACT, 3D row-reduce on DVE
        sq_sb = sqpool.tile([P, cblk, D], FP)
        x2a = x2pool.tile([P, cblk + 1], FP)
        if sq_split:
            h = (SQ_DVE_BLOCKS * cblk) // CBLK
            nc.vector.tensor_mul(sq_sb[:, :h, :], x_sb[:, :h, :], x_sb[:, :h, :])
            nc.scalar.square(
                sq_sb[:, h:, :].rearrange("p c d -> p (c d)"),
                x_sb[:, h:, :].rearrange("p c d -> p (c d)"),
            )
        else:
            nc.vector.tensor_mul(sq_sb, x_sb, x_sb)
        if red_split and sq_split:
            hr = (SQ_DVE_BLOCKS * cblk) // CBLK
            nc.vector.reduce_sum(x2a[:, 0:hr], sq_sb[:, :hr, :],
                                 axis=mybir.AxisListType.X)
            nc.vector.reduce_sum(x2a[:, hr:cblk], sq_sb[:, hr:, :],
                                 axis=mybir.AxisListType.X)
        else:
            nc.vector.reduce_sum(x2a[:, 0:cblk], sq_sb, axis=mybir.AxisListType.X)
        nc.vector.memset(x2a[:, cblk : cblk + 1], 1.0)

        # transpose pairs of blocks: [128, 128] -> psum, copy to SBUF (bf16)
        x_fl = x_sb.rearrange("p c d -> p (c d)")
        xt_sb = xtpool.tile([P, npair, P], w_dt)
        for j in range(npair):
            pst = ps_t.tile([P, P], FP)
            nc.tensor.transpose(pst, x_fl[:, j * P : (j + 1) * P], sb_id)
            if j % 2 == 0:
                nc.scalar.copy(xt_sb[:, j, :], pst)
            else:
                nc.vector.tensor_copy(xt_sb[:, j, :], pst)

        # transpose x2a -> [cblk+1, 128]
        psr = ps_r.tile([cblk + 1, P], FP)
        nc.tensor.transpose(psr, x2a, sb_id)
        x2r = x2pool.tile([cblk + 1, P], ew_dt)
        nc.scalar.copy(x2r, psr)

        # per 512-col psum bank: EW matmul first (zeroes bank, writes x2+w2),
        # then pair matmuls accumulate -2 x.w^T
        o_sb = opool.tile([P, cblk * UNITS], FP)
        ppb = 512 // (2 * UNITS)  # pairs per bank
        for b in range(nbank):
            pso = ps_o.tile([P, 512], FP, tag="pso")
            if ew_last:
                # pair matmuls first (first one zeroes the bank); the EW
                # matmul -- fed by the slower x2 chain -- accumulates last.
                for jj in range(ppb):
                    j = b * ppb + jj
                    nc.tensor.matmul(
                        pso[:, jj * 2 * UNITS : (jj + 1) * 2 * UNITS],
                        lhsT=xt_sb[:, j, :],
                        rhs=sb_w,
                        start=(jj == 0),
                        stop=False,
                        skip_group_check=(jj != 0),
                    )
                nc.tensor.matmul(pso, lhsT=x2r,
                                 rhs=sb_ew[:, b * 512 : (b + 1) * 512],
                                 start=False, stop=True)
            else:
                nc.tensor.matmul(pso, lhsT=x2r,
                                 rhs=sb_ew[:, b * 512 : (b + 1) * 512],
                                 start=True, stop=False)
                for jj in range(ppb):
                    j = b * ppb + jj
                    nc.tensor.matmul(
                        pso[:, jj * 2 * UNITS : (jj + 1) * 2 * UNITS],
                        lhsT=xt_sb[:, j, :],
                        rhs=sb_w,
                        start=False,
                        stop=(jj == ppb - 1),
                        skip_group_check=(jj != ppb - 1),
                    )
            # epilogue for this bank, split ACT/DVE
            ea = EPI_ACT_COLS
            nc.scalar.copy(o_sb[:, b * 512 : b * 512 + ea], pso[:, :ea])
            nc.vector.tensor_copy(o_sb[:, b * 512 + ea : (b + 1) * 512], pso[:, ea:])
        nc.sync.dma_start(ov[t], o_sb)


def build_nc(n_rows: int = N_CORE, ew_mm_f32r: bool = False, repeat: int = 1,
             hw_repeat: int = 1, pair_mm_bf16: bool = False,
             xbar_transpose: bool = False, sq_gpsimd: bool = False,
             big_bufs: bool = False, sq_split: bool = False,
             dma_split: bool = False, cblk: int = CBLK,
             psum_rebal: bool = False, red_split: bool = False,
             ew_last: bool = False):
    nc = bacc.Bacc("TRN2", target_bir_lowering=False, debug=False)
    with tile.TileContext(nc) as tc:
        _knn_tile_kernel(tc, n_rows, ew_mm_f32r, repeat=repeat,
                         hw_repeat=hw_repeat, pair_mm_bf16=pair_mm_bf16,
                         xbar_transpose=xbar_transpose, sq_gpsimd=sq_gpsimd,
                         big_bufs=big_bufs, sq_split=sq_split,
                         dma_split=dma_split, cblk=cblk,
                         psum_rebal=psum_rebal, red_split=red_split,
                         ew_last=ew_last)
    nc.compile()
    return nc


def make_consts(w: np.ndarray, pair_mm_bf16: bool = False, cblk: int = CBLK):
    """Host-side prep of the replicated prototype constants."""
    import ml_dtypes
    w = np.asarray(w, dtype=np.float32)
    wn2 = (-2.0 * w.T).astype(np.float32)          # [64(d), 64(u)]
    wdt = ml_dtypes.bfloat16 if pair_mm_bf16 else np.float32
    wn2t = np.zeros((P, 2 * UNITS), dtype=wdt)  # block-diag
    wn2t[:UNITS, :UNITS] = wn2.astype(wdt)
    wn2t[UNITS:, UNITS:] = wn2.astype(wdt)
    w2 = np.sum(w * w, axis=-1).astype(np.float32)  # [64]
    ew = np.zeros((cblk + 1, cblk * UNITS), dtype=np.float32)
    for c in range(cblk):
        ew[c, c * UNITS : (c + 1) * UNITS] = 1.0
    ew[cblk, :] = np.tile(w2, cblk)
    ident = np.eye(P, dtype=np.float32)
    return {"wn2t": wn2t, "ew": ew, "ident": ident}


_NC_CACHE = {}


def kernel(x: np.ndarray, w: np.ndarray) -> np.ndarray:
    from concourse.bass_utils import run_bass_kernel_spmd

    x = np.asarray(x, dtype=np.float32)
    xf = np.ascontiguousarray(x.reshape(N_TOTAL, D))
    consts = make_consts(w, pair_mm_bf16=True)

    key = ("full", N_CORE)
    if key not in _NC_CACHE:
        _NC_CACHE[key] = build_nc(N_CORE, ew_mm_f32r=True, pair_mm_bf16=True,
                                  big_bufs=True, sq_split=True)
    nc = _NC_CACHE[key]

    in_maps = []
    for i in range(N_CORES):
        shard = xf[i * N_CORE : (i + 1) * N_CORE]
        in_maps.append({"xc": shard, **consts})

    res = run_bass_kernel_spmd(nc, in_maps, core_ids=list(range(N_CORES)))
    parts = [res.results[i]["outc"] for i in range(N_CORES)]
    out = np.concatenate(parts, axis=0).reshape(B, H, W_DIM, UNITS)
    return out


TIMING_BUILD_KWARGS = {"ew_mm_f32r": True, "pair_mm_bf16": True,
                       "big_bufs": True, "sq_split": True}


def build_timing_nc(n_chunks: int = 16, hw_repeat: int = 1, **build_kwargs):
    """NEFF for the perfslope protocol: n_chunks-chunk body inside an
    in-BIR For_i(hw_repeat) loop."""
    return build_nc(n_chunks * CHUNK_ROWS, hw_repeat=hw_repeat, **build_kwargs)


def timing_in_map(n_chunks: int = 16, **build_kwargs):
    rng = np.random.default_rng(0)
    n_rows = n_chunks * CHUNK_ROWS
    xf = rng.standard_normal((n_rows, D)).astype(np.float32)
    w = (rng.standard_normal((UNITS, D)) * 0.05).astype(np.float32)
    consts = make_consts(w, pair_mm_bf16=build_kwargs.get("pair_mm_bf16", False))
    return {"xc": xf, **consts}


if __name__ == "__main__":
    rng = np.random.default_rng(0)
    x = rng.standard_normal((B, H, W_DIM, D), dtype=np.float32)
    w = (rng.standard_normal((UNITS, D)) * 0.05).astype(np.float32)
    out = kernel(x, w)
    x2 = np.sum(x * x, axis=-1, keepdims=True)
    w2 = np.sum(w * w, axis=-1)
    xw = np.einsum("bhwd,ud->bhwu", x, w)
    ref = x2 - 2.0 * xw + w2
    err = np.abs(out - ref).max() / np.abs(ref).max()
    print("rel err:", err)



# revision 24
# speedup vs baseline: 10.7970x; 10.7970x over previous
"""Trainium2 Bass kernel for squared-Euclidean distance to prototypes
(retrieval_knn).

out[b,h,w,u] = ||x[b,h,w,:] - w[u,:]||^2 = x2 - 2*x.w^T + w2

Data-parallel over the flattened row dim (B*H*W = 524288 rows) across 8
NeuronCores, 65536 rows per core, prototypes replicated, no collectives.

The kernel is PE-ingest + HBM bound, so:
- I/O is bf16 both ways (host casts fp32->bf16 in, bf16->fp32 out;
  rel-err budget 2e-2, observed ~5e-3).
- The host pre-packs x d-major with TWO rows per SBUF column:
    xin[t, k, n] = x[row = t*CHUNK + (k>=64)*HALF + n, d = k%64]
  so the data is the MOVING matmul operand and the stationary operand is
  a constant 128x128 block-diagonal matrix; each streamed column carries
  two rows => 1 PE cycle per row per matmul, no transposes, no
  per-block stationary reloads of data:
    mm1: psum[m, n] += sum_k blkdiag(-2w^T)[k,m] * x[k,n]
    mm2: psum[m, n] += sum_k blkdiag(ones)[k,m] * x^2[k,n]
  giving psum[m, n] = -2 x.w + x2 for row-half m//64, u = m%64.
- The +w2[u] term rides the psum->bf16 epilogue for free as a
  per-partition bias (ACT: activation Identity bias; DVE:
  scalar_tensor_tensor add/bypass).
- One contiguous DMA per chunk each way.
"""

import sys
from contextlib import ExitStack, nullcontext

import numpy as np

sys.path.insert(0, "/opt/trn_rl_repo")

import concourse.bass as bass
import concourse.tile as tile
from concourse import bacc, mybir
from concourse._compat import with_exitstack

# Problem geometry (hardcoded per contest contract)
B, H, W_DIM, D = 16, 128, 256, 64
UNITS = 64
N_CORES = 8
N_TOTAL = B * H * W_DIM              # 524288 rows
N_CORE = N_TOTAL // N_CORES          # 65536 rows per core
P = 128                              # partitions

NBANK = 4                            # psum banks (512 cols) per chunk
CHUNK_ROWS = NBANK * 1024            # rows per chunk (2 per column)
N_CHUNKS = N_CORE // CHUNK_ROWS      # 16

FP = mybir.dt.float32
BF = mybir.dt.bfloat16

TIMING_BUILD_KWARGS = {}


@with_exitstack
def _knn_tile_kernel(ctx: ExitStack, tc: tile.TileContext, n_rows: int,
                     hw_repeat: int = 1, nbank: int = NBANK,
                     bufs: int = 12, ps_bufs: int = 8,
                     sq_gp_cols: int = 0, mm_interleave: bool = True,
                     epi_all_act: bool = False,
                     skip_in_dma: bool = False, skip_out_dma: bool = False,
                     skip_sq: bool = False, skip_mm: bool = False,
                     skip_mm2: bool = False, skip_epi: bool = False,
                     dma_mode: str = "sp", sq_mode: str = "dve",
                     epi_mode: str = "act"):
    """Emit the per-core program.

    hw_repeat: wrap the body in a hardware For_i loop re-processing the
    same data N times (timing only; slope over hw_repeat isolates device
    time from axon dispatch overhead).
    sq_gp_cols: columns of each 512-col bank group's square offloaded
    to GPSIMD (taken from the owning engine's range).
    """
    nc = tc.nc
    cols = nbank * 512               # sbuf columns per chunk
    chunk_rows = 2 * cols
    n_chunks = n_rows // chunk_rows
    assert n_rows % chunk_rows == 0

    xin = nc.dram_tensor("xc", [n_chunks, P, cols], BF,
                         kind="ExternalInput").ap()
    out = nc.dram_tensor("outc", [n_chunks, P, cols], BF,
                         kind="ExternalOutput").ap()
    # consts: block-diag(-2w^T), block-diag(ones), w2 column
    wbd = nc.dram_tensor("wbd", [P, P], BF, kind="ExternalInput").ap()
    obd = nc.dram_tensor("obd", [P, P], BF, kind="ExternalInput").ap()
    w2c = nc.dram_tensor("w2c", [P, 1], FP, kind="ExternalInput").ap()

    consts = ctx.enter_context(tc.tile_pool(name="consts", bufs=1))
    sb_wbd = consts.tile([P, P], BF)
    nc.sync.dma_start(sb_wbd, wbd)
    sb_obd = consts.tile([P, P], BF)
    nc.sync.dma_start(sb_obd, obd)
    sb_w2c = consts.tile([P, 1], FP)
    nc.sync.dma_start(sb_w2c, w2c)

    xpool = ctx.enter_context(tc.tile_pool(name="xin", bufs=bufs))
    sqpool = ctx.enter_context(tc.tile_pool(name="sq", bufs=bufs))
    opool = ctx.enter_context(tc.tile_pool(name="osb", bufs=bufs))
    pspool = ctx.enter_context(tc.tile_pool(name="ps", bufs=ps_bufs,
                                            space="PSUM"))

    loop_cm = tc.For_i(0, hw_repeat, 1) if hw_repeat > 1 else nullcontext()
    with loop_cm:
        for t in range(n_chunks):
            x_sb = xpool.tile([P, cols], BF)
            if skip_in_dma:
                # timing ablation: 1/32-size sliver keeps the tile "written"
                nc.sync.dma_start(x_sb[:, :64], xin[t][:, :64])
            elif dma_mode in ("split", "3q"):
                hc = cols // 2
                nc.sync.dma_start(x_sb[:, :hc], xin[t][:, :hc])
                nc.scalar.dma_start(x_sb[:, hc:], xin[t][:, hc:])
            else:
                nc.sync.dma_start(x_sb, xin[t])

            # x^2: per 512-col bank group, alternate DVE/ACT owner;
            # optionally carve sq_gp_cols off each group for GPSIMD.
            if skip_sq:
                sq_sb = x_sb      # timing ablation: mm2 streams x instead
            else:
                sq_sb = sqpool.tile([P, cols], BF)
                for g in range(nbank):
                    lo, hi = g * 512, (g + 1) * 512
                    mid = hi - sq_gp_cols
                    if sq_mode == "dve":
                        eng = "dve"
                    elif sq_mode == "dve_gp":
                        eng = "dve" if g % 2 == 0 else "gp"
                    else:
                        eng = "dve" if g % 2 == 0 else "act"
                    if eng == "dve":
                        nc.vector.tensor_mul(sq_sb[:, lo:mid], x_sb[:, lo:mid],
                                             x_sb[:, lo:mid])
                    elif eng == "gp":
                        nc.gpsimd.tensor_mul(sq_sb[:, lo:mid], x_sb[:, lo:mid],
                                             x_sb[:, lo:mid])
                    else:
                        nc.scalar.square(sq_sb[:, lo:mid], x_sb[:, lo:mid])
                    if sq_gp_cols:
                        nc.gpsimd.tensor_mul(sq_sb[:, mid:hi], x_sb[:, mid:hi],
                                             x_sb[:, mid:hi])

            o_sb = opool.tile([P, cols], BF)
            psos = [pspool.tile([P, 512], FP, tag="pso", name=f"pso{g}")
                    for g in range(nbank)]
            if not skip_mm:
                if mm_interleave:
                    # group same-stationary matmuls to cut LD_WEIGHTS reloads
                    for g in range(nbank):
                        nc.tensor.matmul(psos[g], lhsT=sb_wbd,
                                         rhs=x_sb[:, g * 512:(g + 1) * 512],
                                         start=True, stop=skip_mm2)
                    if not skip_mm2:
                        for g in range(nbank):
                            nc.tensor.matmul(psos[g], lhsT=sb_obd,
                                             rhs=sq_sb[:, g * 512:(g + 1) * 512],
                                             start=False, stop=True,
                                             skip_group_check=True)
                else:
                    for g in range(nbank):
                        nc.tensor.matmul(psos[g], lhsT=sb_wbd,
                                         rhs=x_sb[:, g * 512:(g + 1) * 512],
                                         start=True, stop=skip_mm2)
                        if not skip_mm2:
                            nc.tensor.matmul(psos[g], lhsT=sb_obd,
                                             rhs=sq_sb[:, g * 512:(g + 1) * 512],
                                             start=False, stop=True)

            # psum -> bf16 epilogue with +w2[u] as per-partition bias
            if not skip_epi and not skip_mm:
                for g in range(nbank):
                    ob = o_sb[:, g * 512:(g + 1) * 512]
                    if epi_mode == "dve":
                        on_act = False
                    elif epi_mode == "act":
                        on_act = True
                    else:
                        on_act = (g % 2 == 0)
                    if epi_all_act or on_act:
                        nc.scalar.activation(
                            ob, psos[g],
                            mybir.ActivationFunctionType.Identity,
                            bias=sb_w2c, scale=1.0)
                    else:
                        nc.vector.tensor_scalar_add(ob, psos[g], sb_w2c)
            if not skip_out_dma:
                # in ablation modes o_sb is never written; ship x_sb instead
                src = o_sb if not (skip_epi or skip_mm) else x_sb
                if dma_mode == "out_act":
                    nc.scalar.dma_start(out[t], src)
                elif dma_mode in ("out_pool", "3q"):
                    nc.gpsimd.dma_start(out[t], src)
                elif dma_mode == "split":
                    hc = cols // 2
                    nc.scalar.dma_start(out[t][:, :hc], src[:, :hc])
                    nc.sync.dma_start(out[t][:, hc:], src[:, hc:])
                elif dma_mode == "split_pool":
                    hc = cols // 2
                    nc.gpsimd.dma_start(out[t][:, :hc], src[:, :hc])
                    nc.sync.dma_start(out[t][:, hc:], src[:, hc:])
                else:
                    nc.sync.dma_start(out[t], src)


def build_nc(n_rows: int = N_CORE, hw_repeat: int = 1, **knobs):
    nc = bacc.Bacc("TRN2", target_bir_lowering=False, debug=False)
    with tile.TileContext(nc) as tc:
        _knn_tile_kernel(tc, n_rows, hw_repeat=hw_repeat, **knobs)
    nc.compile()
    return nc


def make_consts(w: np.ndarray):
    """Host-side prep of the replicated prototype constants."""
    import ml_dtypes
    bf = ml_dtypes.bfloat16
    w = np.asarray(w, dtype=np.float32)
    wm2 = -2.0 * w.T                                   # [d, u]
    wbd = np.zeros((P, P), dtype=np.float32)
    wbd[:D, :UNITS] = wm2
    wbd[D:, UNITS:] = wm2
    obd = np.zeros((P, P), dtype=np.float32)
    obd[:D, :UNITS] = 1.0
    obd[D:, UNITS:] = 1.0
    w2 = np.sum(w * w, axis=-1).astype(np.float32)     # [u]
    w2c = np.concatenate([w2, w2]).reshape(P, 1)
    return {"wbd": wbd.astype(bf), "obd": obd.astype(bf), "w2c": w2c}


def pack_x(x: np.ndarray, nbank: int = NBANK):
    """[N_TOTAL, D] fp32 -> per-core [T, 128, cols] bf16: two rows per
    column, d on partitions (d, d+64)."""
    import ml_dtypes
    bf = ml_dtypes.bfloat16
    cols = nbank * 512
    chunk = 2 * cols
    n_chunks = N_CORE // chunk
    xr = x.reshape(N_CORES, n_chunks, 2, cols, D)
    xt = np.ascontiguousarray(xr.transpose(0, 1, 2, 4, 3)).astype(bf)
    return xt.reshape(N_CORES, n_chunks, P, cols)


def unpack_out(res_parts, nbank: int = NBANK):
    """per-core [T, 128, cols] bf16 -> [N_TOTAL, U] fp32."""
    cols = nbank * 512
    chunk = 2 * cols
    n_chunks = N_CORE // chunk
    outs = []
    for arr in res_parts:
        a = arr.reshape(n_chunks, 2, UNITS, cols).transpose(0, 1, 3, 2)
        outs.append(np.ascontiguousarray(a).reshape(N_CORE, UNITS))
    return np.concatenate(outs, axis=0).astype(np.float32)


_NC_CACHE = {}


def kernel(x: np.ndarray, w: np.ndarray) -> np.ndarray:
    from concourse.bass_utils import run_bass_kernel_spmd

    x = np.asarray(x, dtype=np.float32)
    xt = pack_x(x.reshape(N_TOTAL, D))
    consts = make_consts(w)

    key = ("full", N_CORE, NBANK)
    if key not in _NC_CACHE:
        _NC_CACHE[key] = build_nc(N_CORE)
    nc = _NC_CACHE[key]

    in_maps = [{"xc": xt[i], **consts} for i in range(N_CORES)]
    res = run_bass_kernel_spmd(nc, in_maps, core_ids=list(range(N_CORES)))
    out = unpack_out([res.results[i]["outc"] for i in range(N_CORES)])
    return out.reshape(B, H, W_DIM, UNITS)


def build_timing_nc(n_chunks: int = 16, hw_repeat: int = 1, **build_kwargs):
    """NEFF for the perfslope protocol: n_chunks-chunk body inside an
    in-BIR For_i(hw_repeat) loop."""
    nbank = build_kwargs.get("nbank", NBANK)
    return build_nc(n_chunks * 2 * 512 * nbank, hw_repeat=hw_repeat,
                    **build_kwargs)


def timing_in_map(n_chunks: int = 16, **build_kwargs):
    import ml_dtypes
    nbank = build_kwargs.get("nbank", NBANK)
    cols = nbank * 512
    rng = np.random.default_rng(0)
    n_rows = n_chunks * 2 * cols
    xf = rng.standard_normal((n_rows, D)).astype(np.float32)
    w = (rng.standard_normal((UNITS, D)) * 0.05).astype(np.float32)
    xr = xf.reshape(n_chunks, 2, cols, D)
    xt = np.ascontiguousarray(xr.transpose(0, 1, 3, 2)).astype(
        ml_dtypes.bfloat16).reshape(n_chunks, P, cols)
    return {"xc": xt, **make_consts(w)}


if __name__ == "__main__":
    rng = np.random.default_rng(0)
    x = rng.standard_normal((B, H, W_DIM, D), dtype=np.float32)
    w = (rng.standard_normal((UNITS, D)) * 0.05).astype(np.float32)
    out = kernel(x, w)
    x2 = np.sum(x * x, axis=-1, keepdims=True)
    w2 = np.sum(w * w, axis=-1)
    xw = np.einsum("bhwd,ud->bhwu", x, w)
    ref = x2 - 2.0 * xw + w2
    err = np.abs(out - ref).max() / np.abs(ref).max()
    print("rel err:", err)
